# revision 37
# baseline (speedup 1.0000x reference)
"""Compact Bilinear Pooling (count-sketch + FFT circular correlation) as a
Trainium2 Bass kernel, data-parallel over batch across 8 NeuronCores.

Math: FFT(count_sketch(x; s, h))[k] = sum_c x[c] * s[c] * exp(-2pi i h[c] k / D)
    = x @ A, a dense complex matrix built on the host from (s, h). So the whole
layer is: Y1 = X1 @ A1, Y2 = X2 @ A2 (per-row half spectra, fp16 matmuls at
1 col/cycle with automatic fast-weight-load), elementwise complex product +
sum-pool over the 14x14 window (fp16 2x-mode products, a pairwise fp16 fold,
and an fp32 free-axis reduce -- the whole chain on the in-order DVE queue so
it pipelines without cross-engine stalls; PSUM->SBUF fp16 copies on ACT), then
a real inverse FFT of the pooled [4, D] spectrum per core, done fully on-chip
as a two-stage Cooley-Tukey factorization (D = 64*128, k = kt*128 + p) built
from PE transposes + small matmuls -- no DRAM round trip.  The tail runs in
b-pair halves so its first half overlaps the main loop's drain, and a short
dummy-matmul warmup ramps the PE clock while the first DMAs land.
"""
import numpy as np

import concourse.bass as bass
import concourse.tile as tile
from concourse import bacc, mybir
from concourse.bass_utils import run_bass_kernel_spmd

B, Hh, Ww, C, D = 32, 14, 14, 512, 8192
NCORES = 8
BPC = B // NCORES        # 4 batches per core
HW = Hh * Ww             # 196
ROWS = BPC * HW          # 784 rows per core
KT = 33                  # frequency tiles of 128 (KT*128 = 4224 >= D/2 + 1)
KP = KT * 128
CCN = 4                  # contraction chunks (C = 4*128)

F32 = mybir.dt.float32
F32R = mybir.dt.float32r
F16 = mybir.dt.float16


def _round_fp32r(x: np.ndarray) -> np.ndarray:
    """Round-to-nearest-even dropping the low 12 mantissa bits (measured
    float32r behaviour of the DVE rounding path on trn2)."""
    b = np.ascontiguousarray(x, dtype=np.float32).view(np.uint32)
    r = (b + np.uint32(0x7FF) + ((b >> np.uint32(12)) & np.uint32(1))) & np.uint32(0xFFFFF000)
    return r.view(np.float32)


def _build_nc():
    nc = bacc.Bacc("TRN2", target_bir_lowering=False)

    xt_d = nc.dram_tensor("xt", [128, 2, CCN, ROWS], F16, kind="ExternalInput")
    amat_d = nc.dram_tensor("amat", [128, KT, 4, CCN, 128], F16, kind="ExternalInput")
    c33_d = nc.dram_tensor("c33", [KT, 3, 64], F16, kind="ExternalInput")
    tw_d = nc.dram_tensor("tw", [128, 2, BPC, 64], F32, kind="ExternalInput")
    w2_d = nc.dram_tensor("w2", [128, 2, 128], F32R, kind="ExternalInput")
    id_d = nc.dram_tensor("ident", [128, 128], F16, kind="ExternalInput")
    out_d = nc.dram_tensor("out", [BPC, D], F32, kind="ExternalOutput")

    with tile.TileContext(nc) as tc:
        with tc.tile_pool(name="const", bufs=1) as pc, \
             tc.tile_pool(name="astream", bufs=3) as pa, \
             tc.tile_pool(name="ywork", bufs=3) as pyb, \
             tc.tile_pool(name="pwork", bufs=5) as pp:

            # cc0 slice of xt first, then amat kt=0, then the rest of xt --
            # the first matmuls only need (cc=0, kt=0), so they start ~8us in
            xt = pc.tile([128, 2, CCN, ROWS], F16)
            nc.sync.dma_start(xt[:, :, 0], xt_d[:, :, 0])

            # pooled spectrum accumulator [p, kt, term, b]
            qsb = pc.tile([128, KT, 4, 4], F32)

            # warm up the PE clock (pstate ramp) while the first DMAs land --
            # sized to span until the xt/amat tiles arrive (~13us)
            warm = pc.tile([128, 512], F16)
            nc.vector.memset(warm, 0.0)
            with tc.tile_pool(name="pwarm", bufs=1, space="PSUM") as pwm:
                wps = pwm.tile([128, 512], F32)
                for _ in range(12):
                    nc.tensor.matmul(wps, warm[:, :128], warm, start=True, stop=True)
                wsink = pc.tile([128, 1], F32)
                nc.scalar.activation(wsink, wps[:, 0:1],
                                     mybir.ActivationFunctionType.Copy)

            # ---------------- main loop ----------------
            with tc.tile_pool(name="py", bufs=2, space="PSUM") as py:
                for kt in range(KT):
                    at = pa.tile([128, 4, CCN, 128], F16, tag="amat")
                    nc.sync.dma_start(at, amat_d[:, kt])
                    if kt == 0:
                        for cc in range(1, CCN):
                            nc.sync.dma_start(xt[:, :, cc], xt_d[:, :, cc])
                    for rc in range(2):
                        yps = {}
                        for t in range(4):
                            yps[t] = py.tile([128, 392], F32, tag=f"y{t}", name=f"y{t}")
                        for cc in range(CCN):
                            for t in range(4):
                                nc.tensor.matmul(
                                    yps[t],
                                    at[:, t, cc],
                                    xt[:, t // 2, cc, rc * 392:(rc + 1) * 392],
                                    start=(cc == 0),
                                    stop=(cc == CCN - 1),
                                )
                        # PSUM -> SBUF fp16 copies (ACT; GPSIMD cannot read
                        # PSUM).  For the final block, two copies go on DVE so
                        # the ACT/DVE halves run in parallel and the end-of-
                        # loop drain chain is shorter.
                        ysb = pyb.tile([128, 4, 392], F16, tag="ysb")
                        if kt == KT - 1 and rc == 1:
                            nc.scalar.copy(ysb[:, 0], yps[0])
                            nc.scalar.copy(ysb[:, 1], yps[1])
                            nc.vector.tensor_copy(ysb[:, 2], yps[2])
                            nc.vector.tensor_copy(ysb[:, 3], yps[3])
                        else:
                            for t in range(4):
                                nc.scalar.copy(ysb[:, t], yps[t])
                        # products (DVE, all fp16 SBUF -> 2x mode), two terms per
                        # op: (U,V) = (y1r,y1i)*(y2r,y2i) and (T1,T2) =
                        # (y1r,y1i)*(y2i,y2r) via a reversed slice
                        prod = pp.tile([128, 4, 2, 196], F16, tag="prod")
                        nc.vector.tensor_mul(
                            prod[:, 0:2].rearrange("p t s x -> p t (s x)"),
                            ysb[:, 0:2], ysb[:, 2:4])
                        nc.vector.tensor_mul(
                            prod[:, 2:4].rearrange("p t s x -> p t (s x)"),
                            ysb[:, 0:2], ysb[:, 3:1:-1])
                        # pairwise fold 196 -> 98 (DVE, fp16 2x) then fp32
                        # free-axis sum -- whole chain stays on the in-order
                        # DVE queue, so no cross-engine stalls
                        prodF = pp.tile([128, 4, 2, 98], F16, tag="prodF")
                        nc.vector.tensor_add(
                            prodF.rearrange("p t s x -> p (t s) x"),
                            prod[:, :, :, 0:98].rearrange("p t s x -> p (t s) x"),
                            prod[:, :, :, 98:196].rearrange("p t s x -> p (t s) x"))
                        nc.vector.tensor_reduce(
                            qsb[:, kt, :, rc * 2:(rc + 1) * 2],
                            prodF.rearrange("p t s x -> p (t s) x"),
                            axis=mybir.AxisListType.X, op=mybir.AluOpType.add)

            # ---------------- inverse FFT tail (on-chip) ----------------
            # Q[k] with k = kt*128 + p lives as qsb[p, kt].  IFFT via
            # x[t1 + 64*t2] = sum_p e(p t1/8192) e(p t2/128)
            #                   * sum_kt Q[kt*128+p] e(kt t1/64)
            c33 = pc.tile([KT, 3, 64], F16)
            nc.sync.dma_start(c33, c33_d[:, :, :])
            tw = pc.tile([128, 2, BPC, 64], F32)
            nc.sync.dma_start(tw, tw_d[:, :, :, :])
            w2 = pc.tile([128, 2, 128], F32R)
            nc.sync.dma_start(w2, w2_d[:, :, :])
            ident = pc.tile([128, 128], F16)
            nc.sync.dma_start(ident, id_d[:, :])

            with tc.tile_pool(name="tsb", bufs=1) as pt, \
                 tc.tile_pool(name="tps", bufs=1, space="PSUM") as pps, \
                 tc.tile_pool(name="tmm", bufs=1) as pm:
                # combine terms (Qr = U - V, Qi = T1 + T2), transpose Q ->
                # [kt, p], and stage-1 matmuls -- in b-pair halves, so the
                # first half overlaps the last main-loop block's drain
                # (qsb[..., 0:2] is complete one block before qsb[..., 2:4])
                qc = pt.tile([128, 2, KT, 4], F16)
                qt_psr = pps.tile([KT, 4, 128], F16, tag="qtr")
                qt_psi = pps.tile([KT, 4, 128], F16, tag="qti")
                qt_sb = pt.tile([KT, 2, 4, 128], F16)
                wr = pps.tile([64, BPC * 128], F32, tag="wr")
                wi = pps.tile([64, BPC * 128], F32, tag="wi")
                w_sb = pt.tile([64, 2, 4, 128], F16)
                wrt = pps.tile([128, BPC, 64], F16, tag="wrt")
                wit = pps.tile([128, BPC, 64], F16, tag="wit")
                m1 = pm.tile([128, BPC, 64], F32, tag="m1")
                m2 = pm.tile([128, BPC, 64], F32, tag="m2")
                m3 = pm.tile([128, BPC, 64], F32, tag="m3")
                m4 = pm.tile([128, BPC, 64], F32, tag="m4")
                g_sb = pt.tile([128, 2, 4, 64], F32R)
                x_ps = pps.tile([128, BPC * 64], F32, tag="xps")
                res = pt.tile([128, BPC, 64], F32)
                for h in range(2):
                    bs = slice(2 * h, 2 * h + 2)
                    cs = slice(h * 256, (h + 1) * 256)
                    cs_o = slice(h * 128, (h + 1) * 128)
                    nc.vector.tensor_sub(qc[:, 0, :, bs],
                                         qsb[:, :, 0, bs], qsb[:, :, 1, bs])
                    nc.gpsimd.tensor_add(qc[:, 1, :, bs],
                                         qsb[:, :, 2, bs], qsb[:, :, 3, bs])
                    for b in (2 * h, 2 * h + 1):
                        nc.tensor.transpose(qt_psr[:, b], qc[:, 0, :, b], ident)
                        nc.tensor.transpose(qt_psi[:, b], qc[:, 1, :, b], ident)
                    nc.scalar.copy(qt_sb[:, 0, bs], qt_psr[:, bs])
                    nc.scalar.copy(qt_sb[:, 1, bs], qt_psi[:, bs])
                    qr_h = qt_sb[:, 0, bs].rearrange("k b p -> k (b p)")
                    qi_h = qt_sb[:, 1, bs].rearrange("k b p -> k (b p)")
                    nc.tensor.matmul(wr[:, cs], c33[:, 0], qr_h, start=True, stop=False)
                    nc.tensor.matmul(wr[:, cs], c33[:, 2], qi_h, start=False, stop=True)
                    nc.tensor.matmul(wi[:, cs], c33[:, 1], qr_h, start=True, stop=False)
                    nc.tensor.matmul(wi[:, cs], c33[:, 0], qi_h, start=False, stop=True)
                    # W -> SBUF (ACT and DVE in parallel), transpose to
                    # [p, t1], twiddle by e(p t1/8192) -- still per half
                    nc.scalar.copy(w_sb[:, 0, bs], wr[:, cs])
                    nc.vector.tensor_copy(w_sb[:, 1, bs], wi[:, cs])
                    for b in (2 * h, 2 * h + 1):
                        nc.tensor.transpose(wrt[:, b], w_sb[:, 0, b], ident[:64, :64])
                        nc.tensor.transpose(wit[:, b], w_sb[:, 1, b], ident[:64, :64])
                    nc.vector.tensor_mul(m1[:, bs], wrt[:, bs], tw[:, 0, bs])
                    nc.vector.tensor_mul(m2[:, bs], wit[:, bs], tw[:, 1, bs])
                    nc.vector.tensor_mul(m3[:, bs], wrt[:, bs], tw[:, 1, bs])
                    nc.vector.tensor_mul(m4[:, bs], wit[:, bs], tw[:, 0, bs])
                    nc.vector.tensor_sub(g_sb[:, 0, bs], m1[:, bs], m2[:, bs])
                    nc.gpsimd.tensor_add(g_sb[:, 1, bs], m3[:, bs], m4[:, bs])

                    # stage 2 + output store, also per half: the first half's
                    # DMA overlaps the second half's compute
                    nc.tensor.matmul(x_ps[:, cs_o],
                                     w2[:, 0], g_sb[:, 0, bs].rearrange("p b t -> p (b t)"),
                                     start=True, stop=False)
                    nc.tensor.matmul(x_ps[:, cs_o],
                                     w2[:, 1], g_sb[:, 1, bs].rearrange("p b t -> p (b t)"),
                                     start=False, stop=True)
                    nc.scalar.copy(res[:, bs], x_ps[:, cs_o])
                    nc.sync.dma_start(
                        out_d[bs].rearrange("b (t2 t1) -> t2 b t1", t1=64),
                        res[:, bs])

    nc.compile()
    return nc


def _host_consts(rand_s_1, rand_s_2, rand_h_1, rand_h_2):
    k = np.arange(KP)
    alpha = np.where((k == 0) | (k == D // 2), 1.0, 2.0) / D
    alpha = np.where(k > D // 2, 0.0, alpha)
    live = (k <= D // 2).astype(np.float64)
    s1 = rand_s_1.astype(np.float64)
    s2 = rand_s_2.astype(np.float64)
    th1 = 2.0 * np.pi * ((rand_h_1.astype(np.int64)[:, None] * k[None, :]) % D) / D
    th2 = 2.0 * np.pi * ((rand_h_2.astype(np.int64)[:, None] * k[None, :]) % D) / D
    A = np.empty((4, C, KP), np.float32)
    A[0] = s1[:, None] * np.cos(th1) * alpha
    A[1] = -s1[:, None] * np.sin(th1) * alpha
    A[2] = s2[:, None] * np.cos(th2) * live
    A[3] = -s2[:, None] * np.sin(th2) * live
    # amat layout [p, kt, tensor, cc, q]: contiguous 4KB per (p, kt)
    amat = np.ascontiguousarray(
        A.reshape(4, CCN, 128, KT, 128).transpose(2, 3, 0, 1, 4)).astype(np.float16)

    kt_ = np.arange(KT)[:, None]
    t1 = np.arange(64)[None, :]
    c_ = np.cos(2 * np.pi * kt_ * t1 / 64)
    s_ = np.sin(2 * np.pi * kt_ * t1 / 64)
    c33 = np.stack([c_, s_, -s_], 1).astype(np.float16)  # [KT, 3, 64]

    p_ = np.arange(128)[:, None]
    tw = np.stack([np.cos(2 * np.pi * p_ * t1 / D),
                   np.sin(2 * np.pi * p_ * t1 / D)], 1).astype(np.float32)  # [128, 2, 64]
    tw = np.ascontiguousarray(np.repeat(tw[:, :, None, :], BPC, axis=2))  # [128, 2, b, 64]

    t2 = np.arange(128)[None, :]
    w2 = _round_fp32r(np.stack([np.cos(2 * np.pi * p_ * t2 / 128),
                                -np.sin(2 * np.pi * p_ * t2 / 128)],
                               1).astype(np.float32))  # [128, 2, 128]
    ident = np.eye(128, dtype=np.float16)
    return amat, c33, tw, w2, ident


_NC_CACHE = None
LAST_RESULTS = None


def kernel(bottom1, bottom2, rand_s_1, rand_s_2, rand_h_1, rand_h_2):
    global _NC_CACHE
    if _NC_CACHE is None:
        _NC_CACHE = _build_nc()
    nc = _NC_CACHE

    amat, c33, tw, w2, ident = _host_consts(
        np.asarray(rand_s_1), np.asarray(rand_s_2),
        np.asarray(rand_h_1), np.asarray(rand_h_2))

    x1 = np.asarray(bottom1, np.float32).reshape(B, HW, C)
    x2 = np.asarray(bottom2, np.float32).reshape(B, HW, C)

    in_maps = []
    for core in range(NCORES):
        bs = slice(core * BPC, (core + 1) * BPC)
        xt = np.empty((2, C, ROWS), np.float32)
        xt[0] = x1[bs].reshape(ROWS, C).T
        xt[1] = x2[bs].reshape(ROWS, C).T
        xt = np.ascontiguousarray(
            xt.reshape(2, CCN, 128, ROWS).transpose(2, 0, 1, 3)).astype(np.float16)
        in_maps.append({
            "xt": xt, "amat": amat,
            "c33": c33, "tw": tw, "w2": w2, "ident": ident,
        })

    res = run_bass_kernel_spmd(nc, in_maps, core_ids=list(range(NCORES)))
    global LAST_RESULTS
    LAST_RESULTS = res
    out = np.concatenate([res.results[c]["out"] for c in range(NCORES)], 0)
    return out.astype(np.float32)


if __name__ == "__main__":
    rng = np.random.default_rng(0)
    b1 = rng.standard_normal((B, Hh, Ww, C)).astype(np.float32)
    b2 = rng.standard_normal((B, Hh, Ww, C)).astype(np.float32)
    s1 = (2.0 * rng.integers(0, 2, C) - 1.0).astype(np.float32)
    s2 = (2.0 * rng.integers(0, 2, C) - 1.0).astype(np.float32)
    h1 = rng.integers(0, D, C, dtype=np.int32)
    h2 = rng.integers(0, D, C, dtype=np.int32)
    out = kernel(bottom1=b1, bottom2=b2, rand_s_1=s1, rand_s_2=s2,
                 rand_h_1=h1, rand_h_2=h2)
    print(out.shape, out.dtype)


# revision 38
# speedup vs baseline: 1.0032x; 1.0032x over previous
"""Compact Bilinear Pooling (count-sketch + FFT circular correlation) as a
Trainium2 Bass kernel, data-parallel over batch across 8 NeuronCores.

Math: FFT(count_sketch(x; s, h))[k] = sum_c x[c] * s[c] * exp(-2pi i h[c] k / D)
    = x @ A, a dense complex matrix built on the host from (s, h). So the whole
layer is: Y1 = X1 @ A1, Y2 = X2 @ A2 (per-row half spectra, fp16 matmuls at
1 col/cycle with automatic fast-weight-load), elementwise complex product +
sum-pool over the 14x14 window (fp16 2x-mode products, a pairwise fp16 fold,
and an fp32 free-axis reduce -- the whole chain on the in-order DVE queue so
it pipelines without cross-engine stalls; PSUM->SBUF fp16 copies on ACT), then
a real inverse FFT of the pooled [4, D] spectrum per core, done fully on-chip
as a two-stage Cooley-Tukey factorization (D = 64*128, k = kt*128 + p) built
from PE transposes + small matmuls -- no DRAM round trip.  The tail runs in
b-pair halves so its first half overlaps the main loop's drain, and a short
dummy-matmul warmup ramps the PE clock while the first DMAs land.
"""
import numpy as np

import concourse.bass as bass
import concourse.tile as tile
from concourse import bacc, mybir
from concourse.bass_utils import run_bass_kernel_spmd

B, Hh, Ww, C, D = 32, 14, 14, 512, 8192
NCORES = 8
BPC = B // NCORES        # 4 batches per core
HW = Hh * Ww             # 196
ROWS = BPC * HW          # 784 rows per core
KT = 33                  # frequency tiles of 128 (KT*128 = 4224 >= D/2 + 1)
KP = KT * 128
CCN = 4                  # contraction chunks (C = 4*128)

F32 = mybir.dt.float32
F32R = mybir.dt.float32r
F16 = mybir.dt.float16


def _round_fp32r(x: np.ndarray) -> np.ndarray:
    """Round-to-nearest-even dropping the low 12 mantissa bits (measured
    float32r behaviour of the DVE rounding path on trn2)."""
    b = np.ascontiguousarray(x, dtype=np.float32).view(np.uint32)
    r = (b + np.uint32(0x7FF) + ((b >> np.uint32(12)) & np.uint32(1))) & np.uint32(0xFFFFF000)
    return r.view(np.float32)


def _build_nc():
    nc = bacc.Bacc("TRN2", target_bir_lowering=False)

    xt_d = nc.dram_tensor("xt", [128, 2, CCN, ROWS], F16, kind="ExternalInput")
    amat_d = nc.dram_tensor("amat", [128, KT, 4, CCN, 128], F16, kind="ExternalInput")
    c33_d = nc.dram_tensor("c33", [KT, 3, 64], F16, kind="ExternalInput")
    tw_d = nc.dram_tensor("tw", [128, 2, BPC, 64], F32, kind="ExternalInput")
    w2_d = nc.dram_tensor("w2", [128, 2, 128], F32R, kind="ExternalInput")
    id_d = nc.dram_tensor("ident", [128, 128], F16, kind="ExternalInput")
    out_d = nc.dram_tensor("out", [BPC, D], F32, kind="ExternalOutput")

    with tile.TileContext(nc) as tc:
        with tc.tile_pool(name="const", bufs=1) as pc, \
             tc.tile_pool(name="astream", bufs=3) as pa, \
             tc.tile_pool(name="ywork", bufs=3) as pyb, \
             tc.tile_pool(name="pwork", bufs=5) as pp:

            # cc0 slice of xt first, then amat kt=0, then the rest of xt --
            # the first matmuls only need (cc=0, kt=0), so they start ~8us in
            xt = pc.tile([128, 2, CCN, ROWS], F16)
            nc.sync.dma_start(xt[:, :, 0], xt_d[:, :, 0])

            # pooled spectrum accumulator [p, kt, term, b]
            qsb = pc.tile([128, KT, 4, 4], F32)

            # warm up the PE clock (pstate ramp) while the first DMAs land --
            # sized to span until the xt/amat tiles arrive (~13us)
            warm = pc.tile([128, 512], F16)
            nc.vector.memset(warm, 0.0)
            with tc.tile_pool(name="pwarm", bufs=1, space="PSUM") as pwm:
                wps = pwm.tile([128, 512], F32)
                for _ in range(10):
                    nc.tensor.matmul(wps, warm[:, :128], warm, start=True, stop=True)
                wsink = pc.tile([128, 1], F32)
                nc.scalar.activation(wsink, wps[:, 0:1],
                                     mybir.ActivationFunctionType.Copy)

            # ---------------- main loop ----------------
            with tc.tile_pool(name="py", bufs=2, space="PSUM") as py:
                for kt in range(KT):
                    at = pa.tile([128, 4, CCN, 128], F16, tag="amat")
                    if kt == 0:
                        # split the first tile's load per cc: the first
                        # matmuls gate on only the cc=0 slice
                        for cc in range(CCN):
                            nc.sync.dma_start(at[:, :, cc], amat_d[:, kt, :, cc])
                        for cc in range(1, CCN):
                            nc.sync.dma_start(xt[:, :, cc], xt_d[:, :, cc])
                    else:
                        nc.sync.dma_start(at, amat_d[:, kt])
                    for rc in range(2):
                        yps = {}
                        for t in range(4):
                            yps[t] = py.tile([128, 392], F32, tag=f"y{t}", name=f"y{t}")
                        for cc in range(CCN):
                            for t in range(4):
                                nc.tensor.matmul(
                                    yps[t],
                                    at[:, t, cc],
                                    xt[:, t // 2, cc, rc * 392:(rc + 1) * 392],
                                    start=(cc == 0),
                                    stop=(cc == CCN - 1),
                                )
                        # PSUM -> SBUF fp16 copies (ACT; GPSIMD cannot read
                        # PSUM).  For the final block, two copies go on DVE so
                        # the ACT/DVE halves run in parallel and the end-of-
                        # loop drain chain is shorter.
                        ysb = pyb.tile([128, 4, 392], F16, tag="ysb")
                        if kt == KT - 1 and rc == 1:
                            nc.scalar.copy(ysb[:, 0], yps[0])
                            nc.scalar.copy(ysb[:, 1], yps[1])
                            nc.vector.tensor_copy(ysb[:, 2], yps[2])
                            nc.vector.tensor_copy(ysb[:, 3], yps[3])
                        else:
                            for t in range(4):
                                nc.scalar.copy(ysb[:, t], yps[t])
                        # products (DVE, all fp16 SBUF -> 2x mode), two terms per
                        # op: (U,V) = (y1r,y1i)*(y2r,y2i) and (T1,T2) =
                        # (y1r,y1i)*(y2i,y2r) via a reversed slice
                        prod = pp.tile([128, 4, 2, 196], F16, tag="prod")
                        nc.vector.tensor_mul(
                            prod[:, 0:2].rearrange("p t s x -> p t (s x)"),
                            ysb[:, 0:2], ysb[:, 2:4])
                        nc.vector.tensor_mul(
                            prod[:, 2:4].rearrange("p t s x -> p t (s x)"),
                            ysb[:, 0:2], ysb[:, 3:1:-1])
                        # pairwise fold 196 -> 98 (DVE, fp16 2x) then fp32
                        # free-axis sum -- whole chain stays on the in-order
                        # DVE queue, so no cross-engine stalls
                        prodF = pp.tile([128, 4, 2, 98], F16, tag="prodF")
                        nc.vector.tensor_add(
                            prodF.rearrange("p t s x -> p (t s) x"),
                            prod[:, :, :, 0:98].rearrange("p t s x -> p (t s) x"),
                            prod[:, :, :, 98:196].rearrange("p t s x -> p (t s) x"))
                        nc.vector.tensor_reduce(
                            qsb[:, kt, :, rc * 2:(rc + 1) * 2],
                            prodF.rearrange("p t s x -> p (t s) x"),
                            axis=mybir.AxisListType.X, op=mybir.AluOpType.add)

            # ---------------- inverse FFT tail (on-chip) ----------------
            # Q[k] with k = kt*128 + p lives as qsb[p, kt].  IFFT via
            # x[t1 + 64*t2] = sum_p e(p t1/8192) e(p t2/128)
            #                   * sum_kt Q[kt*128+p] e(kt t1/64)
            c33 = pc.tile([KT, 3, 64], F16)
            nc.sync.dma_start(c33, c33_d[:, :, :])
            tw = pc.tile([128, 2, BPC, 64], F32)
            nc.sync.dma_start(tw, tw_d[:, :, :, :])
            w2 = pc.tile([128, 2, 128], F32R)
            nc.sync.dma_start(w2, w2_d[:, :, :])
            ident = pc.tile([128, 128], F16)
            nc.sync.dma_start(ident, id_d[:, :])

            with tc.tile_pool(name="tsb", bufs=1) as pt, \
                 tc.tile_pool(name="tps", bufs=1, space="PSUM") as pps, \
                 tc.tile_pool(name="tmm", bufs=1) as pm:
                # combine terms (Qr = U - V, Qi = T1 + T2), transpose Q ->
                # [kt, p], and stage-1 matmuls -- in b-pair halves, so the
                # first half overlaps the last main-loop block's drain
                # (qsb[..., 0:2] is complete one block before qsb[..., 2:4])
                qc = pt.tile([128, 2, KT, 4], F16)
                qt_psr = pps.tile([KT, 4, 128], F16, tag="qtr")
                qt_psi = pps.tile([KT, 4, 128], F16, tag="qti")
                qt_sb = pt.tile([KT, 2, 4, 128], F16)
                wr = pps.tile([64, BPC * 128], F32, tag="wr")
                wi = pps.tile([64, BPC * 128], F32, tag="wi")
                w_sb = pt.tile([64, 2, 4, 128], F16)
                wrt = pps.tile([128, BPC, 64], F16, tag="wrt")
                wit = pps.tile([128, BPC, 64], F16, tag="wit")
                m1 = pm.tile([128, BPC, 64], F32, tag="m1")
                m2 = pm.tile([128, BPC, 64], F32, tag="m2")
                m3 = pm.tile([128, BPC, 64], F32, tag="m3")
                m4 = pm.tile([128, BPC, 64], F32, tag="m4")
                g_sb = pt.tile([128, 2, 4, 64], F32R)
                x_ps = pps.tile([128, BPC * 64], F32, tag="xps")
                res = pt.tile([128, BPC, 64], F32)
                for h in range(2):
                    bs = slice(2 * h, 2 * h + 2)
                    cs = slice(h * 256, (h + 1) * 256)
                    cs_o = slice(h * 128, (h + 1) * 128)
                    nc.vector.tensor_sub(qc[:, 0, :, bs],
                                         qsb[:, :, 0, bs], qsb[:, :, 1, bs])
                    nc.gpsimd.tensor_add(qc[:, 1, :, bs],
                                         qsb[:, :, 2, bs], qsb[:, :, 3, bs])
                    for b in (2 * h, 2 * h + 1):
                        nc.tensor.transpose(qt_psr[:, b], qc[:, 0, :, b], ident)
                        nc.tensor.transpose(qt_psi[:, b], qc[:, 1, :, b], ident)
                    nc.scalar.copy(qt_sb[:, 0, bs], qt_psr[:, bs])
                    nc.scalar.copy(qt_sb[:, 1, bs], qt_psi[:, bs])
                    qr_h = qt_sb[:, 0, bs].rearrange("k b p -> k (b p)")
                    qi_h = qt_sb[:, 1, bs].rearrange("k b p -> k (b p)")
                    nc.tensor.matmul(wr[:, cs], c33[:, 0], qr_h, start=True, stop=False)
                    nc.tensor.matmul(wr[:, cs], c33[:, 2], qi_h, start=False, stop=True)
                    nc.tensor.matmul(wi[:, cs], c33[:, 1], qr_h, start=True, stop=False)
                    nc.tensor.matmul(wi[:, cs], c33[:, 0], qi_h, start=False, stop=True)
                    # W -> SBUF (ACT and DVE in parallel), transpose to
                    # [p, t1], twiddle by e(p t1/8192) -- still per half
                    nc.scalar.copy(w_sb[:, 0, bs], wr[:, cs])
                    nc.vector.tensor_copy(w_sb[:, 1, bs], wi[:, cs])
                    for b in (2 * h, 2 * h + 1):
                        nc.tensor.transpose(wrt[:, b], w_sb[:, 0, b], ident[:64, :64])
                        nc.tensor.transpose(wit[:, b], w_sb[:, 1, b], ident[:64, :64])
                    nc.vector.tensor_mul(m1[:, bs], wrt[:, bs], tw[:, 0, bs])
                    nc.vector.tensor_mul(m2[:, bs], wit[:, bs], tw[:, 1, bs])
                    nc.vector.tensor_mul(m3[:, bs], wrt[:, bs], tw[:, 1, bs])
                    nc.vector.tensor_mul(m4[:, bs], wit[:, bs], tw[:, 0, bs])
                    nc.vector.tensor_sub(g_sb[:, 0, bs], m1[:, bs], m2[:, bs])
                    nc.gpsimd.tensor_add(g_sb[:, 1, bs], m3[:, bs], m4[:, bs])

                    # stage 2 + output store, also per half: the first half's
                    # DMA overlaps the second half's compute
                    nc.tensor.matmul(x_ps[:, cs_o],
                                     w2[:, 0], g_sb[:, 0, bs].rearrange("p b t -> p (b t)"),
                                     start=True, stop=False)
                    nc.tensor.matmul(x_ps[:, cs_o],
                                     w2[:, 1], g_sb[:, 1, bs].rearrange("p b t -> p (b t)"),
                                     start=False, stop=True)
                    nc.scalar.copy(res[:, bs], x_ps[:, cs_o])
                    nc.sync.dma_start(
                        out_d[bs].rearrange("b (t2 t1) -> t2 b t1", t1=64),
                        res[:, bs])

    nc.compile()
    return nc


def _host_consts(rand_s_1, rand_s_2, rand_h_1, rand_h_2):
    k = np.arange(KP)
    alpha = np.where((k == 0) | (k == D // 2), 1.0, 2.0) / D
    alpha = np.where(k > D // 2, 0.0, alpha)
    live = (k <= D // 2).astype(np.float64)
    s1 = rand_s_1.astype(np.float64)
    s2 = rand_s_2.astype(np.float64)
    th1 = 2.0 * np.pi * ((rand_h_1.astype(np.int64)[:, None] * k[None, :]) % D) / D
    th2 = 2.0 * np.pi * ((rand_h_2.astype(np.int64)[:, None] * k[None, :]) % D) / D
    A = np.empty((4, C, KP), np.float32)
    A[0] = s1[:, None] * np.cos(th1) * alpha
    A[1] = -s1[:, None] * np.sin(th1) * alpha
    A[2] = s2[:, None] * np.cos(th2) * live
    A[3] = -s2[:, None] * np.sin(th2) * live
    # amat layout [p, kt, tensor, cc, q]: contiguous 4KB per (p, kt)
    amat = np.ascontiguousarray(
        A.reshape(4, CCN, 128, KT, 128).transpose(2, 3, 0, 1, 4)).astype(np.float16)

    kt_ = np.arange(KT)[:, None]
    t1 = np.arange(64)[None, :]
    c_ = np.cos(2 * np.pi * kt_ * t1 / 64)
    s_ = np.sin(2 * np.pi * kt_ * t1 / 64)
    c33 = np.stack([c_, s_, -s_], 1).astype(np.float16)  # [KT, 3, 64]

    p_ = np.arange(128)[:, None]
    tw = np.stack([np.cos(2 * np.pi * p_ * t1 / D),
                   np.sin(2 * np.pi * p_ * t1 / D)], 1).astype(np.float32)  # [128, 2, 64]
    tw = np.ascontiguousarray(np.repeat(tw[:, :, None, :], BPC, axis=2))  # [128, 2, b, 64]

    t2 = np.arange(128)[None, :]
    w2 = _round_fp32r(np.stack([np.cos(2 * np.pi * p_ * t2 / 128),
                                -np.sin(2 * np.pi * p_ * t2 / 128)],
                               1).astype(np.float32))  # [128, 2, 128]
    ident = np.eye(128, dtype=np.float16)
    return amat, c33, tw, w2, ident


_NC_CACHE = None
LAST_RESULTS = None


def kernel(bottom1, bottom2, rand_s_1, rand_s_2, rand_h_1, rand_h_2):
    global _NC_CACHE
    if _NC_CACHE is None:
        _NC_CACHE = _build_nc()
    nc = _NC_CACHE

    amat, c33, tw, w2, ident = _host_consts(
        np.asarray(rand_s_1), np.asarray(rand_s_2),
        np.asarray(rand_h_1), np.asarray(rand_h_2))

    x1 = np.asarray(bottom1, np.float32).reshape(B, HW, C)
    x2 = np.asarray(bottom2, np.float32).reshape(B, HW, C)

    in_maps = []
    for core in range(NCORES):
        bs = slice(core * BPC, (core + 1) * BPC)
        xt = np.empty((2, C, ROWS), np.float32)
        xt[0] = x1[bs].reshape(ROWS, C).T
        xt[1] = x2[bs].reshape(ROWS, C).T
        xt = np.ascontiguousarray(
            xt.reshape(2, CCN, 128, ROWS).transpose(2, 0, 1, 3)).astype(np.float16)
        in_maps.append({
            "xt": xt, "amat": amat,
            "c33": c33, "tw": tw, "w2": w2, "ident": ident,
        })

    res = run_bass_kernel_spmd(nc, in_maps, core_ids=list(range(NCORES)))
    global LAST_RESULTS
    LAST_RESULTS = res
    out = np.concatenate([res.results[c]["out"] for c in range(NCORES)], 0)
    return out.astype(np.float32)


if __name__ == "__main__":
    rng = np.random.default_rng(0)
    b1 = rng.standard_normal((B, Hh, Ww, C)).astype(np.float32)
    b2 = rng.standard_normal((B, Hh, Ww, C)).astype(np.float32)
    s1 = (2.0 * rng.integers(0, 2, C) - 1.0).astype(np.float32)
    s2 = (2.0 * rng.integers(0, 2, C) - 1.0).astype(np.float32)
    h1 = rng.integers(0, D, C, dtype=np.int32)
    h2 = rng.integers(0, D, C, dtype=np.int32)
    out = kernel(bottom1=b1, bottom2=b2, rand_s_1=s1, rand_s_2=s2,
                 rand_h_1=h1, rand_h_2=h2)
    print(out.shape, out.dtype)


# revision 39
# speedup vs baseline: 1.0211x; 1.0179x over previous
"""Compact Bilinear Pooling (count-sketch + FFT circular correlation) as a
Trainium2 Bass kernel, data-parallel over batch across 8 NeuronCores.

Math: FFT(count_sketch(x; s, h))[k] = sum_c x[c] * s[c] * exp(-2pi i h[c] k / D)
    = x @ A, a dense complex matrix built on the host from (s, h). So the whole
layer is: Y1 = X1 @ A1, Y2 = X2 @ A2 (per-row half spectra, fp16 matmuls at
1 col/cycle with automatic fast-weight-load), elementwise complex product +
sum-pool over the 14x14 window (fp16 2x-mode products, a pairwise fp16 fold,
and an fp32 free-axis reduce -- the whole chain on the in-order DVE queue so
it pipelines without cross-engine stalls; PSUM->SBUF fp16 copies on ACT), then
a real inverse FFT of the pooled [4, D] spectrum per core, done fully on-chip
as a two-stage Cooley-Tukey factorization (D = 64*128, k = kt*128 + p) built
from PE transposes + small matmuls -- no DRAM round trip.  The tail runs in
b-pair halves so its first half overlaps the main loop's drain, and a short
dummy-matmul warmup ramps the PE clock while the first DMAs land.
"""
import numpy as np

import concourse.bass as bass
import concourse.tile as tile
from concourse import bacc, mybir
from concourse.bass_utils import run_bass_kernel_spmd

B, Hh, Ww, C, D = 32, 14, 14, 512, 8192
NCORES = 8
BPC = B // NCORES        # 4 batches per core
HW = Hh * Ww             # 196
ROWS = BPC * HW          # 784 rows per core
KT = 33                  # frequency tiles of 128 (KT*128 = 4224 >= D/2 + 1)
KP = KT * 128
CCN = 4                  # contraction chunks (C = 4*128)

F32 = mybir.dt.float32
F32R = mybir.dt.float32r
F16 = mybir.dt.float16


def _round_fp32r(x: np.ndarray) -> np.ndarray:
    """Round-to-nearest-even dropping the low 12 mantissa bits (measured
    float32r behaviour of the DVE rounding path on trn2)."""
    b = np.ascontiguousarray(x, dtype=np.float32).view(np.uint32)
    r = (b + np.uint32(0x7FF) + ((b >> np.uint32(12)) & np.uint32(1))) & np.uint32(0xFFFFF000)
    return r.view(np.float32)


def _build_nc():
    nc = bacc.Bacc("TRN2", target_bir_lowering=False)

    xt_d = nc.dram_tensor("xt", [128, 2, CCN, ROWS], F16, kind="ExternalInput")
    amat_d = nc.dram_tensor("amat", [128, KT, 4, CCN, 128], F16, kind="ExternalInput")
    c33_d = nc.dram_tensor("c33", [KT, 3, 64], F16, kind="ExternalInput")
    tw_d = nc.dram_tensor("tw", [128, 2, BPC, 64], F32, kind="ExternalInput")
    w2_d = nc.dram_tensor("w2", [128, 2, 128], F32R, kind="ExternalInput")
    id_d = nc.dram_tensor("ident", [128, 128], F16, kind="ExternalInput")
    out_d = nc.dram_tensor("out", [BPC, D], F32, kind="ExternalOutput")

    with tile.TileContext(nc) as tc:
        with tc.tile_pool(name="const", bufs=1) as pc, \
             tc.tile_pool(name="astream", bufs=3) as pa, \
             tc.tile_pool(name="ywork", bufs=3) as pyb, \
             tc.tile_pool(name="pwork", bufs=5) as pp:

            # cc0 slice of xt first, then amat kt=0, then the rest of xt --
            # the first matmuls only need (cc=0, kt=0), so they start ~8us in
            xt = pc.tile([128, 2, CCN, ROWS], F16)
            nc.sync.dma_start(xt[:, :, 0], xt_d[:, :, 0])

            # pooled spectrum accumulator [p, kt, term, b]
            qsb = pc.tile([128, KT, 4, 4], F32)

            # warm up the PE clock (pstate ramp) while the first DMAs land --
            # sized to span until the xt/amat tiles arrive (~13us)
            warm = pc.tile([128, 512], F16)
            nc.vector.memset(warm, 0.0)
            with tc.tile_pool(name="pwarm", bufs=1, space="PSUM") as pwm:
                wps = pwm.tile([128, 512], F32)
                for _ in range(10):
                    nc.tensor.matmul(wps, warm[:, :128], warm, start=True, stop=True)
                wsink = pc.tile([128, 1], F32)
                nc.scalar.activation(wsink, wps[:, 0:1],
                                     mybir.ActivationFunctionType.Copy)

            # ---------------- main loop ----------------
            with tc.tile_pool(name="py", bufs=2, space="PSUM") as py:
                for kt in range(KT):
                    at = pa.tile([128, 4, CCN, 128], F16, tag="amat")
                    if kt == 0:
                        # split the first tile's load per cc: the first
                        # matmuls gate on only the cc=0 slice
                        for cc in range(CCN):
                            nc.sync.dma_start(at[:, :, cc], amat_d[:, kt, :, cc])
                        for cc in range(1, CCN):
                            nc.sync.dma_start(xt[:, :, cc], xt_d[:, :, cc])
                    else:
                        nc.sync.dma_start(at, amat_d[:, kt])
                    for rc in range(2):
                        yps = {}
                        for t in range(4):
                            yps[t] = py.tile([128, 392], F32, tag=f"y{t}", name=f"y{t}")
                        # t-major: each y-tile's accumulation closes 4 matmuls
                        # apart, so its PSUM->SBUF copy overlaps the rest of
                        # this block's matmuls instead of queueing after them
                        for t in range(4):
                            for cc in range(CCN):
                                nc.tensor.matmul(
                                    yps[t],
                                    at[:, t, cc],
                                    xt[:, t // 2, cc, rc * 392:(rc + 1) * 392],
                                    start=(cc == 0),
                                    stop=(cc == CCN - 1),
                                )
                        # PSUM -> SBUF fp16 copies (ACT; GPSIMD cannot read
                        # PSUM).  For the final block, two copies go on DVE so
                        # the ACT/DVE halves run in parallel and the end-of-
                        # loop drain chain is shorter.
                        ysb = pyb.tile([128, 4, 392], F16, tag="ysb")
                        if kt == KT - 1 and rc == 1:
                            nc.scalar.copy(ysb[:, 0], yps[0])
                            nc.scalar.copy(ysb[:, 1], yps[1])
                            nc.vector.tensor_copy(ysb[:, 2], yps[2])
                            nc.vector.tensor_copy(ysb[:, 3], yps[3])
                        else:
                            for t in range(4):
                                nc.scalar.copy(ysb[:, t], yps[t])
                        # products (DVE, all fp16 SBUF -> 2x mode), two terms per
                        # op: (U,V) = (y1r,y1i)*(y2r,y2i) and (T1,T2) =
                        # (y1r,y1i)*(y2i,y2r) via a reversed slice
                        prod = pp.tile([128, 4, 2, 196], F16, tag="prod")
                        nc.vector.tensor_mul(
                            prod[:, 0:2].rearrange("p t s x -> p t (s x)"),
                            ysb[:, 0:2], ysb[:, 2:4])
                        nc.vector.tensor_mul(
                            prod[:, 2:4].rearrange("p t s x -> p t (s x)"),
                            ysb[:, 0:2], ysb[:, 3:1:-1])
                        # pairwise fold 196 -> 98 (DVE, fp16 2x) then fp32
                        # free-axis sum -- whole chain stays on the in-order
                        # DVE queue, so no cross-engine stalls
                        prodF = pp.tile([128, 4, 2, 98], F16, tag="prodF")
                        nc.vector.tensor_add(
                            prodF.rearrange("p t s x -> p (t s) x"),
                            prod[:, :, :, 0:98].rearrange("p t s x -> p (t s) x"),
                            prod[:, :, :, 98:196].rearrange("p t s x -> p (t s) x"))
                        nc.vector.tensor_reduce(
                            qsb[:, kt, :, rc * 2:(rc + 1) * 2],
                            prodF.rearrange("p t s x -> p (t s) x"),
                            axis=mybir.AxisListType.X, op=mybir.AluOpType.add)

            # ---------------- inverse FFT tail (on-chip) ----------------
            # Q[k] with k = kt*128 + p lives as qsb[p, kt].  IFFT via
            # x[t1 + 64*t2] = sum_p e(p t1/8192) e(p t2/128)
            #                   * sum_kt Q[kt*128+p] e(kt t1/64)
            c33 = pc.tile([KT, 3, 64], F16)
            nc.sync.dma_start(c33, c33_d[:, :, :])
            tw = pc.tile([128, 2, BPC, 64], F32)
            nc.sync.dma_start(tw, tw_d[:, :, :, :])
            w2 = pc.tile([128, 2, 128], F32R)
            nc.sync.dma_start(w2, w2_d[:, :, :])
            ident = pc.tile([128, 128], F16)
            nc.sync.dma_start(ident, id_d[:, :])

            with tc.tile_pool(name="tsb", bufs=1) as pt, \
                 tc.tile_pool(name="tps", bufs=1, space="PSUM") as pps, \
                 tc.tile_pool(name="tmm", bufs=1) as pm:
                # combine terms (Qr = U - V, Qi = T1 + T2), transpose Q ->
                # [kt, p], and stage-1 matmuls -- in b-pair halves, so the
                # first half overlaps the last main-loop block's drain
                # (qsb[..., 0:2] is complete one block before qsb[..., 2:4])
                qc = pt.tile([128, 2, KT, 4], F16)
                qt_psr = pps.tile([KT, 4, 128], F16, tag="qtr")
                qt_psi = pps.tile([KT, 4, 128], F16, tag="qti")
                qt_sb = pt.tile([KT, 2, 4, 128], F16)
                wr = pps.tile([64, BPC * 128], F32, tag="wr")
                wi = pps.tile([64, BPC * 128], F32, tag="wi")
                w_sb = pt.tile([64, 2, 4, 128], F16)
                wrt = pps.tile([128, BPC, 64], F16, tag="wrt")
                wit = pps.tile([128, BPC, 64], F16, tag="wit")
                m1 = pm.tile([128, BPC, 64], F32, tag="m1")
                m2 = pm.tile([128, BPC, 64], F32, tag="m2")
                m3 = pm.tile([128, BPC, 64], F32, tag="m3")
                m4 = pm.tile([128, BPC, 64], F32, tag="m4")
                g_sb = pt.tile([128, 2, 4, 64], F32R)
                x_ps = pps.tile([128, BPC * 64], F32, tag="xps")
                res = pt.tile([128, BPC, 64], F32)
                for h in range(2):
                    bs = slice(2 * h, 2 * h + 2)
                    cs = slice(h * 256, (h + 1) * 256)
                    cs_o = slice(h * 128, (h + 1) * 128)
                    nc.vector.tensor_sub(qc[:, 0, :, bs],
                                         qsb[:, :, 0, bs], qsb[:, :, 1, bs])
                    nc.gpsimd.tensor_add(qc[:, 1, :, bs],
                                         qsb[:, :, 2, bs], qsb[:, :, 3, bs])
                    for b in (2 * h, 2 * h + 1):
                        nc.tensor.transpose(qt_psr[:, b], qc[:, 0, :, b], ident)
                        nc.tensor.transpose(qt_psi[:, b], qc[:, 1, :, b], ident)
                    nc.scalar.copy(qt_sb[:, 0, bs], qt_psr[:, bs])
                    nc.scalar.copy(qt_sb[:, 1, bs], qt_psi[:, bs])
                    qr_h = qt_sb[:, 0, bs].rearrange("k b p -> k (b p)")
                    qi_h = qt_sb[:, 1, bs].rearrange("k b p -> k (b p)")
                    nc.tensor.matmul(wr[:, cs], c33[:, 0], qr_h, start=True, stop=False)
                    nc.tensor.matmul(wr[:, cs], c33[:, 2], qi_h, start=False, stop=True)
                    nc.tensor.matmul(wi[:, cs], c33[:, 1], qr_h, start=True, stop=False)
                    nc.tensor.matmul(wi[:, cs], c33[:, 0], qi_h, start=False, stop=True)
                    # W -> SBUF (ACT and DVE in parallel), transpose to
                    # [p, t1], twiddle by e(p t1/8192) -- still per half
                    nc.scalar.copy(w_sb[:, 0, bs], wr[:, cs])
                    nc.vector.tensor_copy(w_sb[:, 1, bs], wi[:, cs])
                    for b in (2 * h, 2 * h + 1):
                        nc.tensor.transpose(wrt[:, b], w_sb[:, 0, b], ident[:64, :64])
                        nc.tensor.transpose(wit[:, b], w_sb[:, 1, b], ident[:64, :64])
                    nc.vector.tensor_mul(m1[:, bs], wrt[:, bs], tw[:, 0, bs])
                    nc.vector.tensor_mul(m2[:, bs], wit[:, bs], tw[:, 1, bs])
                    nc.vector.tensor_mul(m3[:, bs], wrt[:, bs], tw[:, 1, bs])
                    nc.vector.tensor_mul(m4[:, bs], wit[:, bs], tw[:, 0, bs])
                    nc.vector.tensor_sub(g_sb[:, 0, bs], m1[:, bs], m2[:, bs])
                    nc.gpsimd.tensor_add(g_sb[:, 1, bs], m3[:, bs], m4[:, bs])

                    # stage 2 + output store, also per half: the first half's
                    # DMA overlaps the second half's compute
                    nc.tensor.matmul(x_ps[:, cs_o],
                                     w2[:, 0], g_sb[:, 0, bs].rearrange("p b t -> p (b t)"),
                                     start=True, stop=False)
                    nc.tensor.matmul(x_ps[:, cs_o],
                                     w2[:, 1], g_sb[:, 1, bs].rearrange("p b t -> p (b t)"),
                                     start=False, stop=True)
                    nc.scalar.copy(res[:, bs], x_ps[:, cs_o])
                    nc.sync.dma_start(
                        out_d[bs].rearrange("b (t2 t1) -> t2 b t1", t1=64),
                        res[:, bs])

    nc.compile()
    return nc


def _host_consts(rand_s_1, rand_s_2, rand_h_1, rand_h_2):
    k = np.arange(KP)
    alpha = np.where((k == 0) | (k == D // 2), 1.0, 2.0) / D
    alpha = np.where(k > D // 2, 0.0, alpha)
    live = (k <= D // 2).astype(np.float64)
    s1 = rand_s_1.astype(np.float64)
    s2 = rand_s_2.astype(np.float64)
    th1 = 2.0 * np.pi * ((rand_h_1.astype(np.int64)[:, None] * k[None, :]) % D) / D
    th2 = 2.0 * np.pi * ((rand_h_2.astype(np.int64)[:, None] * k[None, :]) % D) / D
    A = np.empty((4, C, KP), np.float32)
    A[0] = s1[:, None] * np.cos(th1) * alpha
    A[1] = -s1[:, None] * np.sin(th1) * alpha
    A[2] = s2[:, None] * np.cos(th2) * live
    A[3] = -s2[:, None] * np.sin(th2) * live
    # amat layout [p, kt, tensor, cc, q]: contiguous 4KB per (p, kt)
    amat = np.ascontiguousarray(
        A.reshape(4, CCN, 128, KT, 128).transpose(2, 3, 0, 1, 4)).astype(np.float16)

    kt_ = np.arange(KT)[:, None]
    t1 = np.arange(64)[None, :]
    c_ = np.cos(2 * np.pi * kt_ * t1 / 64)
    s_ = np.sin(2 * np.pi * kt_ * t1 / 64)
    c33 = np.stack([c_, s_, -s_], 1).astype(np.float16)  # [KT, 3, 64]

    p_ = np.arange(128)[:, None]
    tw = np.stack([np.cos(2 * np.pi * p_ * t1 / D),
                   np.sin(2 * np.pi * p_ * t1 / D)], 1).astype(np.float32)  # [128, 2, 64]
    tw = np.ascontiguousarray(np.repeat(tw[:, :, None, :], BPC, axis=2))  # [128, 2, b, 64]

    t2 = np.arange(128)[None, :]
    w2 = _round_fp32r(np.stack([np.cos(2 * np.pi * p_ * t2 / 128),
                                -np.sin(2 * np.pi * p_ * t2 / 128)],
                               1).astype(np.float32))  # [128, 2, 128]
    ident = np.eye(128, dtype=np.float16)
    return amat, c33, tw, w2, ident


_NC_CACHE = None
LAST_RESULTS = None


def kernel(bottom1, bottom2, rand_s_1, rand_s_2, rand_h_1, rand_h_2):
    global _NC_CACHE
    if _NC_CACHE is None:
        _NC_CACHE = _build_nc()
    nc = _NC_CACHE

    amat, c33, tw, w2, ident = _host_consts(
        np.asarray(rand_s_1), np.asarray(rand_s_2),
        np.asarray(rand_h_1), np.asarray(rand_h_2))

    x1 = np.asarray(bottom1, np.float32).reshape(B, HW, C)
    x2 = np.asarray(bottom2, np.float32).reshape(B, HW, C)

    in_maps = []
    for core in range(NCORES):
        bs = slice(core * BPC, (core + 1) * BPC)
        xt = np.empty((2, C, ROWS), np.float32)
        xt[0] = x1[bs].reshape(ROWS, C).T
        xt[1] = x2[bs].reshape(ROWS, C).T
        xt = np.ascontiguousarray(
            xt.reshape(2, CCN, 128, ROWS).transpose(2, 0, 1, 3)).astype(np.float16)
        in_maps.append({
            "xt": xt, "amat": amat,
            "c33": c33, "tw": tw, "w2": w2, "ident": ident,
        })

    res = run_bass_kernel_spmd(nc, in_maps, core_ids=list(range(NCORES)))
    global LAST_RESULTS
    LAST_RESULTS = res
    out = np.concatenate([res.results[c]["out"] for c in range(NCORES)], 0)
    return out.astype(np.float32)


if __name__ == "__main__":
    rng = np.random.default_rng(0)
    b1 = rng.standard_normal((B, Hh, Ww, C)).astype(np.float32)
    b2 = rng.standard_normal((B, Hh, Ww, C)).astype(np.float32)
    s1 = (2.0 * rng.integers(0, 2, C) - 1.0).astype(np.float32)
    s2 = (2.0 * rng.integers(0, 2, C) - 1.0).astype(np.float32)
    h1 = rng.integers(0, D, C, dtype=np.int32)
    h2 = rng.integers(0, D, C, dtype=np.int32)
    out = kernel(bottom1=b1, bottom2=b2, rand_s_1=s1, rand_s_2=s2,
                 rand_h_1=h1, rand_h_2=h2)
    print(out.shape, out.dtype)
